# revision 51
# baseline (speedup 1.0000x reference)
"""Windowed-attention transformer layer on 8 trn2 NeuronCores.

Sharding: the 4096 (B=2 x L=2048) token rows are split into 8 contiguous
chunks of 512 (4 per batch element). Each core gets its chunk plus a
128-token halo per side (window 256), zero-padded at batch edges, and
recomputes LN1+QKV on the halo -> fully independent cores, no collectives.

Structure:
- QKV GEMM and the attention-output projection run in FP8 (e4m3) with
  DoubleRow perf mode (2 contraction rows per PE cell -> 2x matmul
  throughput).  Weights are scaled by 64 on the host to clear the e4m3
  denormal range; the scale is divided back out through the softmax
  normalization (ones column = 8 -> avT holds 8x values in fp8) and a
  1/512 factor fused into the residual add.  FFN stays bf16 (fp8 there
  would eat most of the 2e-2 error budget).
- v is computed in NATURAL layout straight from the QKV GEMM, with a
  per-head ones column appended so the attention AV matmul also produces
  the softmax denominator (augmented-V trick).
- attention scores are computed TRANSPOSED (keys on partitions) so the
  exp output feeds the AV matmul directly -- no PSUM->SBUF bounce of the
  attention weights.
- the banded window mask is ADDED ON THE PE as an accumulating matmul
  (identity stationary x mask moving) per 128-key chunk.  One PSUM bank
  sustains only one live accumulation group, so each chunk's QK+mask
  group is closed before the next chunk starts.
- softmax normalization: reciprocal of the matmul-produced sums, a K=1
  ones-matmul broadcast into spare columns of the same PSUM bank, and a
  multiply fused into the PSUM->SBUF copy.
- QKV GEMM and attention are interleaved per head-pair so PE/Act/DVE all
  stay busy; LN statistics, softmax sums, residuals are fp32.

LN gains/biases and linear biases are identities per the input spec and
are skipped.
"""

import numpy as np
import ml_dtypes

import concourse.bass as bass
import concourse.tile as tile
from concourse import mybir
from concourse.bass_utils import run_bass_kernel_spmd
from concourse.vector_clock import ScopedClock, VectorClock
from concourse.tile_scheduler import N_PROCS

F32 = mybir.dt.float32
BF16 = mybir.dt.bfloat16
F8 = mybir.dt.float8e4
AF = mybir.ActivationFunctionType
ALU = mybir.AluOpType
DR = mybir.MatmulPerfMode.DoubleRow

B, L, D = 2, 2048, 1024
H, HD = 16, 64
R = 768          # local rows incl. halo
OWN = 512        # owned rows per core
HALO = 128
NEG = -1.0e9
WS = 64.0        # host-side fp8 weight scale for wq/wo
ONEC = 8.0       # vna ones column: makes avT = 8 * av (fp8 range), 64/8=8
EXPS = 0.125 / (WS * WS)   # exp scale absorbs q,k both carrying x64


class SplitWaitTileContext(tile.TileContext):
    """Walrus in this container allows at most ONE sync wait per
    instruction: split extra waits onto preceding same-engine NoOps, and
    emit the tail drain as one drain per outstanding proc."""
    _ctr = 0

    def _add_instruction(self, inst):
        si = inst.sync_info
        if si is not None and si.on_wait and len(si.on_wait) > 1:
            waits = list(si.on_wait)
            for w in waits[:-1]:
                SplitWaitTileContext._ctr += 1
                nop = mybir.InstNoOp(name=f"splitw-{SplitWaitTileContext._ctr}", ins=[], outs=[])
                nop.engine = inst.engine
                nop.sync_info = mybir.SyncInfo(on_wait=[w], on_update=[])
                super()._add_instruction(nop)
            inst.sync_info = mybir.SyncInfo(on_wait=[waits[-1]], on_update=list(si.on_update))
        super()._add_instruction(inst)

    def _drain_and_barrier(self, tick_clock, wait_clock):
        gc = tick_clock.global_clock
        for p in range(N_PROCS):
            if gc[p] > 0:
                vals = [0] * N_PROCS
                vals[p] = gc[p]
                d = self.nc.sync.drain()
                wait_clock.add_sem_waits(d.ins, ScopedClock({None: VectorClock(vals)}))
        self.nc.sync.drain()
        self.nc.all_engine_barrier()
        assert self.sems is not None
        popped = self.nc._tile_sem_poison_stack.pop()
        assert popped is self._sem_poison
        self.nc.clear_and_free_semaphores(list(self.sems.allocated().values()))
        self.nc.all_engine_barrier()


# ---------------------------------------------------------------------------
# device program (identical on all 8 cores; only input data differs)
# ---------------------------------------------------------------------------
_CACHED = {}


def _build_program():
    if "nc" in _CACHED:
        return _CACHED["nc"]

    nc = bass.Bass("TRN2", target_bir_lowering=False, debug=False, num_devices=1)

    xs = nc.dram_tensor("xs", [R, D], F32, kind="ExternalInput").ap()
    # fp8 DoubleRow pair layouts: [pair, 128, 2*cols]
    wq8 = nc.dram_tensor("wq8", [4, 128, 2 * 3 * D], F8, kind="ExternalInput").ap()
    wo8 = nc.dram_tensor("wo8", [4, 128, 2 * D], F8, kind="ExternalInput").ap()
    w18 = nc.dram_tensor("w18", [4, 128, 2 * 2 * D], F8, kind="ExternalInput").ap()
    w2 = nc.dram_tensor("w2", [2 * D, D], BF16, kind="ExternalInput").ap()
    ident_d = nc.dram_tensor("ident", [128, 128], BF16, kind="ExternalInput").ap()
    mask_d = nc.dram_tensor("maskd", [3, 128, 384], BF16, kind="ExternalInput").ap()
    out_d = nc.dram_tensor("out", [OWN, D], F32, kind="ExternalOutput").ap()

    cp = [0]  # copy engine round-robin

    def copy(dst, src):
        cp[0] ^= 1
        if cp[0]:
            nc.vector.tensor_copy(dst, src)
        else:
            nc.scalar.copy(dst, src)

    with SplitWaitTileContext(nc) as tc:
        with (
            tc.tile_pool(name="per", bufs=1) as per,      # persistent
            tc.tile_pool(name="xq", bufs=6) as xq,        # x tiles (fp32)
            tc.tile_pool(name="work", bufs=3) as work,    # h tiles / out tiles
            tc.tile_pool(name="attn", bufs=6) as attn,    # small LN/attention tiles
            tc.tile_pool(name="wts", bufs=16) as wts,     # streamed weights 2KB class
            tc.tile_pool(name="w1p", bufs=4) as w1p,      # ffn_w1 chunks 4KB class
            tc.tile_pool(name="ps", bufs=1, space="PSUM") as ps,
        ):
            # x tiles first on the SP queue so phase A starts ASAP
            xts = []
            for t in range(6):
                xt = xq.tile([128, D], F32, tag="xt", name=f"xpre{t}")
                nc.sync.dma_start(xt[:], xs[t * 128:(t + 1) * 128, :])
                xts.append(xt)
            ident = per.tile([128, 128], BF16, tag="ident")
            nc.gpsimd.dma_start(ident[:], ident_d[:])
            masks = []
            for i in range(3):
                m = per.tile([128, 384], BF16, tag=f"mask{i}")
                nc.gpsimd.dma_start(m[:], mask_d[i])
                masks.append(m)
            mask_for_qb = [masks[0], masks[1], masks[1], masks[2]]

            epsb = per.tile([128, 1], F32, tag="epsb")
            nc.vector.memset(epsb[:], 1e-5)
            ones64 = per.tile([1, 64], BF16, tag="ones64")
            nc.vector.memset(ones64[:], 1.0)


            # persistent activations
            hTp = [per.tile([128, 2, R], F8, tag=f"hTp{c}", name=f"hTp{c}") for c in range(4)]
            qT = [per.tile([128, OWN], BF16, tag=f"qT{d}", name=f"qT{d}") for d in range(8)]
            kT = [per.tile([128, R], BF16, tag=f"kT{d}", name=f"kT{d}") for d in range(8)]
            vna = [per.tile([128, 1040], BF16, tag=f"vna{t}", name=f"vna{t}") for t in range(6)]
            avTp = [per.tile([128, 2, OWN], F8, tag=f"avTp{c}", name=f"avTp{c}") for c in range(4)]
            x2 = [per.tile([128, D], F32, tag=f"x2_{t}", name=f"x2_{t}") for t in range(4)]
            h2Tp = [per.tile([128, 2, OWN], F8, tag=f"h2Tp{c}", name=f"h2Tp{c}") for c in range(4)]
            gT = [per.tile([128, OWN], BF16, tag=f"gT{m}", name=f"gT{m}") for m in range(16)]

            # weight loads on the SP queue (after the x tiles above)
            def wload(cols, src, dt=F8):
                t = wts.tile([128, cols], dt, tag="wchunk", name="wt")
                nc.sync.dma_start(t[:], src)
                return t

            def wsec(sec):
                # pair tiles [128, 2, 1024] of wq8 section sec (q=0, k=1, v=2)
                out = []
                for c in range(4):
                    w = wq8[c].rearrange("p (i n) -> p i n", i=2)[:, :, sec * D:(sec + 1) * D]
                    t = wts.tile([128, 2, D], F8, tag="wchunk", name="wt")
                    nc.sync.dma_start(t[:], w)
                    out.append(t)
                return out

            wv = wsec(2)
            wqs = wsec(0)

            # ---- Phase A: LN1 + transpose -> hTp (fp8) ----
            def layernorm_tile(xt, h):
                st = attn.tile([128, 12], F32, tag="st")
                nc.vector.bn_stats(st[:, 0:6], xt[:, 0:512])
                nc.vector.bn_stats(st[:, 6:12], xt[:, 512:1024])
                mv = attn.tile([128, 2], F32, tag="mv")
                nc.vector.bn_aggr(mv[:], st[:].rearrange("p (g s) -> p g s", g=2))
                std = attn.tile([128, 1], F32, tag="std")
                nc.scalar.activation(std[:], mv[:, 1:2], AF.Sqrt, bias=epsb[:])
                rstd = attn.tile([128, 1], F32, tag="rstd")
                nc.vector.reciprocal(rstd[:], std[:])
                negmu = attn.tile([128, 1], F32, tag="negmu")
                nc.vector.tensor_scalar(out=negmu[:], in0=mv[:, 0:1], scalar1=-1.0,
                                        scalar2=None, op0=ALU.mult)
                neg = attn.tile([128, 1], F32, tag="neg")
                nc.vector.tensor_scalar(out=neg[:], in0=negmu[:], scalar1=rstd[:],
                                        scalar2=None, op0=ALU.mult)
                nc.scalar.activation(h[:], xt[:], AF.Identity, bias=neg[:], scale=rstd[:])

            for t in range(6):
                xt = xts[t]
                h = work.tile([128, D], BF16, tag="h")
                layernorm_tile(xt, h)
                for d in range(8):
                    pt = ps.tile([128, 128], BF16, tag="sc", bufs=3, name="pt")
                    nc.tensor.transpose(pt[:], h[:, d * 128:(d + 1) * 128], ident[:])
                    copy(hTp[d // 2][:, d % 2, t * 128:(t + 1) * 128], pt[:])
                # ---- Phase B-v for this tile: v natural with ones column ----
                # (values carry x64 from the weight scale; ones = 8 so the
                #  softmax normalization leaves avT = 8 * av for fp8 range)
                ones_cols = vna[t].rearrange("p (h x) -> p h x", x=65)[:, :, 64:65]
                nc.vector.memset(ones_cols, ONEC)
                for nh in range(2):
                    pv = ps.tile([128, 512], F32, tag="pa", bufs=2, name="pv")
                    for c in range(4):
                        nc.tensor.matmul(pv[:], hTp[c][:, :, t * 128:(t + 1) * 128],
                                         wv[c][:, :, nh * 512:(nh + 1) * 512],
                                         start=(c == 0), stop=(c == 3), perf_mode=DR)
                    dst = vna[t].rearrange("p (h x) -> p h x", x=65)[:, nh * 8:(nh + 1) * 8, 0:64]
                    copy(dst, pv[:].rearrange("p (h d) -> p h d", d=64))

            wk = wsec(1)

            # ---- Phase B q/k (all head-pairs) ----
            for p in range(8):
                # q m-tile p: heads 2p, 2p+1; own tokens only
                pq = ps.tile([128, 512], F32, tag="pa", bufs=2, name="pq")
                for c in range(4):
                    nc.tensor.matmul(pq[:], wqs[c][:, :, p * 128:(p + 1) * 128],
                                     hTp[c][:, :, HALO:HALO + OWN],
                                     start=(c == 0), stop=(c == 3), perf_mode=DR)
                copy(qT[p][:], pq[:])
                # k m-tile p: all R halo tokens, two half-passes
                for half in range(2):
                    pk = ps.tile([128, 384], F32, tag="pa", bufs=2, name="pk")
                    for c in range(4):
                        nc.tensor.matmul(pk[:], wk[c][:, :, p * 128:(p + 1) * 128],
                                         hTp[c][:, :, half * 384:(half + 1) * 384],
                                         start=(c == 0), stop=(c == 3), perf_mode=DR)
                    copy(kT[p][:, half * 384:(half + 1) * 384], pk[:])

            # prefetch out-proj / ffn1 weights while attention runs
            wos = []
            for c in range(4):
                wt = wts.tile([128, 2, D], F8, tag="wchunk", name="wt")
                nc.sync.dma_start(wt[:], wo8[c].rearrange("p (i n) -> p i n", i=2))
                wos.append(wt)
            w1s = []
            for c in range(4):
                wt = w1p.tile([128, 2, 2 * D], F8, tag="w1c", name="wt")
                nc.sync.dma_start(wt[:], w18[c].rearrange("p (i n) -> p i n", i=2))
                w1s.append(wt)

            # ---- Phase D attention (query-block outer), E/F interleaved:
            #      after query block qb completes for all heads, that token
            #      tile's out-proj + residual + LN2 runs in D's engine gaps
            def emit_ef(t):
                # out-proj + residual + LN2 + transpose for token tile t
                xo = xq.tile([128, D], F32, tag="xt", name="xo")
                nc.sync.dma_start(xo[:], xs[HALO + t * 128:HALO + (t + 1) * 128, :])
                for nh in range(2):
                    po = ps.tile([128, 512], F32, tag="pa", bufs=2, name="po")
                    for c in range(4):
                        nc.tensor.matmul(po[:], avTp[c][:, :, t * 128:(t + 1) * 128],
                                         wos[c][:, :, nh * 512:(nh + 1) * 512],
                                         start=(c == 0), stop=(c == 3), perf_mode=DR)
                    # x2 = po / (8 * 64) + x   (avT carries x8, wo carries x64)
                    nc.vector.scalar_tensor_tensor(
                        out=x2[t][:, nh * 512:(nh + 1) * 512], in0=po[:],
                        scalar=1.0 / (ONEC * WS), in1=xo[:, nh * 512:(nh + 1) * 512],
                        op0=ALU.mult, op1=ALU.add)
                h2 = work.tile([128, D], BF16, tag="h2")
                layernorm_tile(x2[t], h2)
                for d in range(8):
                    pt = ps.tile([128, 128], BF16, tag="sc", bufs=3, name="pt2")
                    nc.tensor.transpose(pt[:], h2[:, d * 128:(d + 1) * 128], ident[:])
                    copy(h2Tp[d // 2][:, d % 2, t * 128:(t + 1) * 128], pt[:])

            def finalize_pair(p, qb, pavt, rs):
                # deferred softmax-normalize of pair (qb, p): runs one pair
                # behind so the PE bcast never stalls on the DVE reciprocal
                for s in range(2):
                    nc.tensor.matmul(pavt[0:64, 256 + s * 128:256 + (s + 1) * 128],
                                     ones64[:], rs[:, s * 128:(s + 1) * 128],
                                     start=True, stop=True)
                    rsb = attn.tile([64, 128], BF16, tag="rsb", bufs=4)
                    copy(rsb[:], pavt[0:64, 256 + s * 128:256 + (s + 1) * 128])
                    nc.vector.tensor_tensor(
                        out=avTp[p // 2][s * 64:(s + 1) * 64, p % 2,
                                         qb * 128:(qb + 1) * 128],
                        in0=pavt[0:64, s * 128:(s + 1) * 128], in1=rsb[:],
                        op=ALU.mult)

            for qb in range(4):
                for p in range(8):
                    exts = []
                    for s in range(2):
                        # one PSUM bank supports a single live accumulation
                        # group on HW: close each chunk's group (QK + mask
                        # add) before the next chunk's start re-arms the bank
                        sct = ps.tile([128, 384], F32, tag="sc", bufs=3, name="sct")
                        for c in range(3):
                            kc = kT[p][s * 64:s * 64 + 64,
                                       qb * 128 + c * 128:qb * 128 + (c + 1) * 128]
                            qs = qT[p][s * 64:s * 64 + 64, qb * 128:(qb + 1) * 128]
                            nc.tensor.matmul(sct[:, c * 128:(c + 1) * 128], kc, qs,
                                             start=True, stop=False)
                            nc.tensor.matmul(sct[:, c * 128:(c + 1) * 128], ident[:],
                                             mask_for_qb[qb][:, c * 128:(c + 1) * 128],
                                             start=False, stop=True)
                        ext = attn.tile([128, 384], BF16, tag="exT", bufs=6)
                        nc.scalar.activation(ext[:], sct[:], AF.Exp, bias=0.0, scale=EXPS)
                        exts.append(ext)
                    pavt = ps.tile([128, 512], F32, tag="pav", bufs=3, name="pavt")
                    for s in range(2):
                        hh = 2 * p + s
                        for c in range(3):
                            nc.tensor.matmul(pavt[0:65, s * 128:(s + 1) * 128],
                                             vna[qb + c][:, hh * 65:hh * 65 + 65],
                                             exts[s][:, c * 128:(c + 1) * 128],
                                             start=(c == 0), stop=(c == 2))
                    # softmax 1/sum for both subs in one op (sum rows adjacent)
                    rs = attn.tile([1, 256], BF16, tag="rs", bufs=8)
                    with nc.allow_low_precision(reason="softmax 1/sum in bf16"):
                        nc.vector.reciprocal(rs[:], pavt[64:65, 0:256])
                    finalize_pair(p, qb, pavt, rs)

                # E/F one block behind D so its matmuls never stall the PE
                # queue on D(qb)'s normalization tail
                if qb >= 1:
                    emit_ef(qb - 1)
            emit_ef(3)

            # ---- Phase G: FFN (bf16) ----
            for m in range(16):
                pg = ps.tile([128, 512], F32, tag="pa", bufs=2, name="pg")
                for c in range(4):
                    nc.tensor.matmul(pg[:], w1s[c][:, :, m * 128:(m + 1) * 128],
                                     h2Tp[c][:, :, :],
                                     start=(c == 0), stop=(c == 3), perf_mode=DR)
                # gelu(pg / 64): undo the fp8 weight scale exactly
                nc.scalar.activation(gT[m][:], pg[:], AF.Gelu, scale=1.0 / WS)

            w2s = [wload(D, w2[k * 128:(k + 1) * 128, :], BF16) for k in range(16)]
            for t in range(4):
                ot = work.tile([128, D], F32, tag="ot", bufs=2)
                for nh in range(2):
                    po = ps.tile([128, 512], F32, tag="pa", bufs=2, name="po2")
                    for k in range(16):
                        nc.tensor.matmul(po[:], gT[k][:, t * 128:(t + 1) * 128],
                                         w2s[k][:, nh * 512:(nh + 1) * 512],
                                         start=(k == 0), stop=(k == 15))
                    nc.vector.tensor_tensor(out=ot[:, nh * 512:(nh + 1) * 512],
                                            in0=po[:], in1=x2[t][:, nh * 512:(nh + 1) * 512],
                                            op=ALU.add)
                nc.sync.dma_start(out_d[t * 128:(t + 1) * 128, :], ot[:])

    _CACHED["nc"] = nc
    return nc


# ---------------------------------------------------------------------------
# host wrapper
# ---------------------------------------------------------------------------
def _pair8(w, scale):
    """[K, N] f32 -> [K//256, 128, 2*N] e4m3 DoubleRow pair layout."""
    f8 = ml_dtypes.float8_e4m3
    K, N = w.shape
    w8 = (np.asarray(w, np.float32) * scale).astype(f8)
    return np.ascontiguousarray(
        w8.reshape(K // 256, 2, 128, N).transpose(0, 2, 1, 3).reshape(K // 256, 128, 2 * N))


def _host_inputs(x, qkv_w, out_w, ffn_w1, ffn_w2):
    bf = ml_dtypes.bfloat16
    shared = {
        "wq8": _pair8(qkv_w, WS),
        "wo8": _pair8(out_w, WS),
        "w18": _pair8(ffn_w1, WS),
        "w2": np.ascontiguousarray(np.asarray(ffn_w2).astype(bf)),
        "ident": np.eye(128, dtype=bf),
    }
    r = np.arange(128)
    # transposed-score masks [key_local, query]: for query i, keys j in
    # [i, i+256] of the 384-band are valid
    t_lo = np.where(r[:, None] >= r[None, :], 0.0, NEG).astype(np.float32)
    t_hi = np.where(r[:, None] <= r[None, :], 0.0, NEG).astype(np.float32)
    zeros = np.zeros((128, 128), np.float32)
    full = np.full((128, 128), NEG, np.float32)

    def band(c0, c2):
        return np.concatenate([c0, zeros, c2], axis=1)

    in_maps = []
    for core in range(8):
        b, ck = core // 4, core % 4
        lo = ck * 512 - HALO
        xsl = np.zeros((R, D), np.float32)
        s, e = max(lo, 0), min(lo + R, L)
        xsl[s - lo:e - lo] = x[b, s:e]
        m_first = band(full if ck == 0 else t_lo, t_hi)
        m_mid = band(t_lo, t_hi)
        m_last = band(t_lo, full if ck == 3 else t_hi)
        in_maps.append({
            "xs": xsl,
            "maskd": np.stack([m_first, m_mid, m_last]).astype(bf),
            **shared,
        })
    return in_maps


def kernel(x, qkv_w, qkv_b, out_w, out_b, ln1_g, ln1_b, ln2_g, ln2_b,
           ffn_w1, ffn_b1, ffn_w2, ffn_b2, _return_results=False):
    x = np.asarray(x, np.float32)
    nc = _build_program()
    in_maps = _host_inputs(x, np.asarray(qkv_w), np.asarray(out_w),
                           np.asarray(ffn_w1), np.asarray(ffn_w2))
    res = run_bass_kernel_spmd(nc, in_maps, list(range(8)))
    out = np.empty((B, L, D), np.float32)
    for core in range(8):
        b, ck = core // 4, core % 4
        out[b, ck * 512:(ck + 1) * 512] = res.results[core]["out"]
    if _return_results:
        return out, res
    return out


# revision 60
# speedup vs baseline: 1.0433x; 1.0433x over previous
"""Windowed-attention transformer layer on 8 trn2 NeuronCores.

Sharding: the 4096 (B=2 x L=2048) token rows are split into 8 contiguous
chunks of 512 (4 per batch element). Each core gets its chunk plus a
128-token halo per side (window 256), zero-padded at batch edges, and
recomputes LN1+QKV on the halo -> fully independent cores, no collectives.

Structure:
- QKV GEMM and the attention-output projection run in FP8 (e4m3) with
  DoubleRow perf mode (2 contraction rows per PE cell -> 2x matmul
  throughput).  Weights are scaled by 64 on the host to clear the e4m3
  denormal range; the scale is divided back out through the softmax
  normalization (ones column = 8 -> avT holds 8x values in fp8) and a
  1/512 factor fused into the residual add.  FFN stays bf16 (fp8 there
  would eat most of the 2e-2 error budget).
- v is computed in NATURAL layout straight from the QKV GEMM, with a
  per-head ones column appended so the attention AV matmul also produces
  the softmax denominator (augmented-V trick).
- attention scores are computed TRANSPOSED (keys on partitions) so the
  exp output feeds the AV matmul directly -- no PSUM->SBUF bounce of the
  attention weights.
- the banded window mask is ADDED ON THE PE as an accumulating matmul
  (identity stationary x mask moving) per 128-key chunk.  One PSUM bank
  sustains only one live accumulation group, so each chunk's QK+mask
  group is closed before the next chunk starts.
- softmax normalization: reciprocal of the matmul-produced sums, a K=1
  ones-matmul broadcast into spare columns of the same PSUM bank, and a
  multiply fused into the PSUM->SBUF copy.
- QKV GEMM and attention are interleaved per head-pair so PE/Act/DVE all
  stay busy; LN statistics, softmax sums, residuals are fp32.

LN gains/biases and linear biases are identities per the input spec and
are skipped.
"""

import numpy as np
import ml_dtypes

import concourse.bass as bass
import concourse.tile as tile
from concourse import mybir
from concourse.bass_utils import run_bass_kernel_spmd
from concourse.vector_clock import ScopedClock, VectorClock
from concourse.tile_scheduler import N_PROCS

F32 = mybir.dt.float32
BF16 = mybir.dt.bfloat16
F8 = mybir.dt.float8e4
AF = mybir.ActivationFunctionType
ALU = mybir.AluOpType
DR = mybir.MatmulPerfMode.DoubleRow

B, L, D = 2, 2048, 1024
H, HD = 16, 64
R = 768          # local rows incl. halo
OWN = 512        # owned rows per core
HALO = 128
NEG = -1.0e9
WS = 64.0        # host-side fp8 weight scale for wq/wo
ONEC = 8.0       # vna ones column: makes avT = 8 * av (fp8 range), 64/8=8
EXPS = 0.125 / (WS * WS)   # exp scale absorbs q,k both carrying x64


class SplitWaitTileContext(tile.TileContext):
    """Walrus in this container allows at most ONE sync wait per
    instruction: split extra waits onto preceding same-engine NoOps, and
    emit the tail drain as one drain per outstanding proc."""
    _ctr = 0

    def _add_instruction(self, inst):
        si = inst.sync_info
        if si is not None and si.on_wait and len(si.on_wait) > 1:
            waits = list(si.on_wait)
            for w in waits[:-1]:
                SplitWaitTileContext._ctr += 1
                nop = mybir.InstNoOp(name=f"splitw-{SplitWaitTileContext._ctr}", ins=[], outs=[])
                nop.engine = inst.engine
                nop.sync_info = mybir.SyncInfo(on_wait=[w], on_update=[])
                super()._add_instruction(nop)
            inst.sync_info = mybir.SyncInfo(on_wait=[waits[-1]], on_update=list(si.on_update))
        super()._add_instruction(inst)

    def _drain_and_barrier(self, tick_clock, wait_clock):
        gc = tick_clock.global_clock
        for p in range(N_PROCS):
            if gc[p] > 0:
                vals = [0] * N_PROCS
                vals[p] = gc[p]
                d = self.nc.sync.drain()
                wait_clock.add_sem_waits(d.ins, ScopedClock({None: VectorClock(vals)}))
        self.nc.sync.drain()
        self.nc.all_engine_barrier()
        assert self.sems is not None
        popped = self.nc._tile_sem_poison_stack.pop()
        assert popped is self._sem_poison
        self.nc.clear_and_free_semaphores(list(self.sems.allocated().values()))
        self.nc.all_engine_barrier()


# ---------------------------------------------------------------------------
# device program (identical on all 8 cores; only input data differs)
# ---------------------------------------------------------------------------
_CACHED = {}


def _build_program():
    if "nc" in _CACHED:
        return _CACHED["nc"]

    nc = bass.Bass("TRN2", target_bir_lowering=False, debug=False, num_devices=1)

    xs = nc.dram_tensor("xs", [R, D], F32, kind="ExternalInput").ap()
    # fp8 DoubleRow pair layouts: [pair, 128, 2*cols]
    wq8 = nc.dram_tensor("wq8", [4, 128, 2 * 3 * D], F8, kind="ExternalInput").ap()
    wo8 = nc.dram_tensor("wo8", [4, 128, 2 * D], F8, kind="ExternalInput").ap()
    w18 = nc.dram_tensor("w18", [4, 128, 2 * 2 * D], F8, kind="ExternalInput").ap()
    w2 = nc.dram_tensor("w2", [2 * D, D], BF16, kind="ExternalInput").ap()
    ident_d = nc.dram_tensor("ident", [128, 128], BF16, kind="ExternalInput").ap()
    mask_d = nc.dram_tensor("maskd", [3, 128, 384], BF16, kind="ExternalInput").ap()
    out_d = nc.dram_tensor("out", [OWN, D], F32, kind="ExternalOutput").ap()

    cp = [0]  # copy engine round-robin

    def copy(dst, src):
        cp[0] ^= 1
        if cp[0]:
            nc.vector.tensor_copy(dst, src)
        else:
            nc.scalar.copy(dst, src)

    with SplitWaitTileContext(nc) as tc:
        with (
            tc.tile_pool(name="per", bufs=1) as per,      # persistent
            tc.tile_pool(name="xq", bufs=6) as xq,        # x tiles (fp32)
            tc.tile_pool(name="work", bufs=3) as work,    # h tiles / out tiles
            tc.tile_pool(name="attn", bufs=6) as attn,    # small LN/attention tiles
            tc.tile_pool(name="wts", bufs=16) as wts,     # streamed weights 2KB class
            tc.tile_pool(name="w1p", bufs=4) as w1p,      # ffn_w1 chunks 4KB class
            tc.tile_pool(name="ps", bufs=1, space="PSUM") as ps,
        ):
            # x tiles first on the SP queue so phase A starts ASAP
            xts = []
            for t in range(6):
                xt = xq.tile([128, D], F32, tag="xt", name=f"xpre{t}")
                nc.sync.dma_start(xt[:], xs[t * 128:(t + 1) * 128, :])
                xts.append(xt)
            ident = per.tile([128, 128], BF16, tag="ident")
            nc.gpsimd.dma_start(ident[:], ident_d[:])
            masks = []
            for i in range(3):
                m = per.tile([128, 384], BF16, tag=f"mask{i}")
                nc.gpsimd.dma_start(m[:], mask_d[i])
                masks.append(m)
            mask_for_qb = [masks[0], masks[1], masks[1], masks[2]]

            epsb = per.tile([128, 1], F32, tag="epsb")
            nc.vector.memset(epsb[:], 1e-5)
            ones64 = per.tile([1, 64], BF16, tag="ones64")
            nc.vector.memset(ones64[:], 1.0)


            # persistent activations
            hTp = [per.tile([128, 2, R], F8, tag=f"hTp{c}", name=f"hTp{c}") for c in range(4)]
            qT = [per.tile([128, OWN], BF16, tag=f"qT{d}", name=f"qT{d}") for d in range(8)]
            kT = [per.tile([128, R], BF16, tag=f"kT{d}", name=f"kT{d}") for d in range(8)]
            vna = [per.tile([128, 1040], BF16, tag=f"vna{t}", name=f"vna{t}") for t in range(6)]
            avTp = [per.tile([128, 2, OWN], F8, tag=f"avTp{c}", name=f"avTp{c}") for c in range(4)]
            x2 = [per.tile([128, D], F32, tag=f"x2_{t}", name=f"x2_{t}") for t in range(4)]
            h2Tp = [per.tile([128, 2, OWN], F8, tag=f"h2Tp{c}", name=f"h2Tp{c}") for c in range(4)]
            gT = [per.tile([128, OWN], BF16, tag=f"gT{m}", name=f"gT{m}") for m in range(16)]

            # weight loads on the SP queue (after the x tiles above)
            def wload(cols, src, dt=F8):
                t = wts.tile([128, cols], dt, tag="wchunk", name="wt")
                nc.sync.dma_start(t[:], src)
                return t

            def wsec(sec):
                # pair tiles [128, 2, 1024] of wq8 section sec (q=0, k=1, v=2)
                out = []
                for c in range(4):
                    w = wq8[c].rearrange("p (i n) -> p i n", i=2)[:, :, sec * D:(sec + 1) * D]
                    t = wts.tile([128, 2, D], F8, tag="wchunk", name="wt")
                    nc.sync.dma_start(t[:], w)
                    out.append(t)
                return out

            wv = wsec(2)
            wqs = wsec(0)

            # ---- Phase A: LN1 + transpose -> hTp (fp8) ----
            def layernorm_tile(xt, h):
                st = attn.tile([128, 12], F32, tag="st")
                nc.vector.bn_stats(st[:, 0:6], xt[:, 0:512])
                nc.vector.bn_stats(st[:, 6:12], xt[:, 512:1024])
                mv = attn.tile([128, 2], F32, tag="mv")
                nc.vector.bn_aggr(mv[:], st[:].rearrange("p (g s) -> p g s", g=2))
                std = attn.tile([128, 1], F32, tag="std")
                nc.scalar.activation(std[:], mv[:, 1:2], AF.Sqrt, bias=epsb[:])
                rstd = attn.tile([128, 1], F32, tag="rstd")
                nc.vector.reciprocal(rstd[:], std[:])
                negmu = attn.tile([128, 1], F32, tag="negmu")
                nc.vector.tensor_scalar(out=negmu[:], in0=mv[:, 0:1], scalar1=-1.0,
                                        scalar2=None, op0=ALU.mult)
                neg = attn.tile([128, 1], F32, tag="neg")
                nc.vector.tensor_scalar(out=neg[:], in0=negmu[:], scalar1=rstd[:],
                                        scalar2=None, op0=ALU.mult)
                nc.scalar.activation(h[:], xt[:], AF.Identity, bias=neg[:], scale=rstd[:])

            for t in range(6):
                xt = xts[t]
                h = work.tile([128, D], BF16, tag="h")
                layernorm_tile(xt, h)
                for d in range(8):
                    pt = ps.tile([128, 128], BF16, tag="sc", bufs=3, name="pt")
                    nc.tensor.transpose(pt[:], h[:, d * 128:(d + 1) * 128], ident[:])
                    copy(hTp[d // 2][:, d % 2, t * 128:(t + 1) * 128], pt[:])
                # ---- Phase B-v for this tile: v natural with ones column ----
                # (values carry x64 from the weight scale; ones = 8 so the
                #  softmax normalization leaves avT = 8 * av for fp8 range)
                ones_cols = vna[t].rearrange("p (h x) -> p h x", x=65)[:, :, 64:65]
                nc.vector.memset(ones_cols, ONEC)
                for nh in range(2):
                    pv = ps.tile([128, 512], F32, tag="pa", bufs=2, name="pv")
                    for c in range(4):
                        nc.tensor.matmul(pv[:], hTp[c][:, :, t * 128:(t + 1) * 128],
                                         wv[c][:, :, nh * 512:(nh + 1) * 512],
                                         start=(c == 0), stop=(c == 3), perf_mode=DR)
                    dst = vna[t].rearrange("p (h x) -> p h x", x=65)[:, nh * 8:(nh + 1) * 8, 0:64]
                    copy(dst, pv[:].rearrange("p (h d) -> p h d", d=64))

            wk = wsec(1)

            # ---- Phase B q/k (all head-pairs) ----
            for p in range(8):
                # q m-tile p: heads 2p, 2p+1; own tokens only
                pq = ps.tile([128, 512], F32, tag="sc", bufs=3, name="pq")
                for c in range(4):
                    nc.tensor.matmul(pq[:], wqs[c][:, :, p * 128:(p + 1) * 128],
                                     hTp[c][:, :, HALO:HALO + OWN],
                                     start=(c == 0), stop=(c == 3), perf_mode=DR)
                copy(qT[p][:], pq[:])
                # k m-tile p: all R halo tokens, two half-passes
                for half in range(2):
                    pk = ps.tile([128, 384], F32, tag="sc", bufs=3, name="pk")
                    for c in range(4):
                        nc.tensor.matmul(pk[:], wk[c][:, :, p * 128:(p + 1) * 128],
                                         hTp[c][:, :, half * 384:(half + 1) * 384],
                                         start=(c == 0), stop=(c == 3), perf_mode=DR)
                    copy(kT[p][:, half * 384:(half + 1) * 384], pk[:])

            # prefetch out-proj / ffn1 weights while attention runs
            wos = []
            for c in range(4):
                wt = wts.tile([128, 2, D], F8, tag="wchunk", name="wt")
                nc.sync.dma_start(wt[:], wo8[c].rearrange("p (i n) -> p i n", i=2))
                wos.append(wt)
            w1s = []
            for c in range(4):
                wt = w1p.tile([128, 2, 2 * D], F8, tag="w1c", name="wt")
                nc.sync.dma_start(wt[:], w18[c].rearrange("p (i n) -> p i n", i=2))
                w1s.append(wt)

            # ---- Phase D attention (query-block outer), E/F interleaved:
            #      after query block qb completes for all heads, that token
            #      tile's out-proj + residual + LN2 runs in D's engine gaps
            def emit_ef(t):
                # out-proj + residual + LN2 + transpose for token tile t
                xo = xq.tile([128, D], F32, tag="xt", name="xo")
                nc.sync.dma_start(xo[:], xs[HALO + t * 128:HALO + (t + 1) * 128, :])
                for nh in range(2):
                    po = ps.tile([128, 512], F32, tag="pa", bufs=2, name="po")
                    for c in range(4):
                        nc.tensor.matmul(po[:], avTp[c][:, :, t * 128:(t + 1) * 128],
                                         wos[c][:, :, nh * 512:(nh + 1) * 512],
                                         start=(c == 0), stop=(c == 3), perf_mode=DR)
                    # x2 = po / (8 * 64) + x   (avT carries x8, wo carries x64)
                    nc.vector.scalar_tensor_tensor(
                        out=x2[t][:, nh * 512:(nh + 1) * 512], in0=po[:],
                        scalar=1.0 / (ONEC * WS), in1=xo[:, nh * 512:(nh + 1) * 512],
                        op0=ALU.mult, op1=ALU.add)
                h2 = work.tile([128, D], BF16, tag="h2")
                layernorm_tile(x2[t], h2)
                for d in range(8):
                    pt = ps.tile([128, 128], BF16, tag="sc", bufs=3, name="pt2")
                    nc.tensor.transpose(pt[:], h2[:, d * 128:(d + 1) * 128], ident[:])
                    copy(h2Tp[d // 2][:, d % 2, t * 128:(t + 1) * 128], pt[:])

            def finalize_pair(p, qb, pavt, rs):
                # deferred softmax-normalize of pair (qb, p): runs one pair
                # behind so the PE bcast never stalls on the DVE reciprocal
                for s in range(2):
                    nc.tensor.matmul(pavt[0:64, 256 + s * 128:256 + (s + 1) * 128],
                                     ones64[:], rs[:, s * 128:(s + 1) * 128],
                                     start=True, stop=True)
                    rsb = attn.tile([64, 128], BF16, tag="rsb", bufs=4)
                    copy(rsb[:], pavt[0:64, 256 + s * 128:256 + (s + 1) * 128])
                    nc.vector.tensor_tensor(
                        out=avTp[p // 2][s * 64:(s + 1) * 64, p % 2,
                                         qb * 128:(qb + 1) * 128],
                        in0=pavt[0:64, s * 128:(s + 1) * 128], in1=rsb[:],
                        op=ALU.mult)

            for qb in range(4):
                for p in range(8):
                    exts = []
                    for s in range(2):
                        # one PSUM bank supports a single live accumulation
                        # group on HW: close each chunk's group (QK + mask
                        # add) before the next chunk's start re-arms the bank
                        sct = ps.tile([128, 384], F32, tag="sc", bufs=3, name="sct")
                        for c in range(3):
                            kc = kT[p][s * 64:s * 64 + 64,
                                       qb * 128 + c * 128:qb * 128 + (c + 1) * 128]
                            qs = qT[p][s * 64:s * 64 + 64, qb * 128:(qb + 1) * 128]
                            nc.tensor.matmul(sct[:, c * 128:(c + 1) * 128], kc, qs,
                                             start=True, stop=False)
                            nc.tensor.matmul(sct[:, c * 128:(c + 1) * 128], ident[:],
                                             mask_for_qb[qb][:, c * 128:(c + 1) * 128],
                                             start=False, stop=True)
                        ext = attn.tile([128, 384], BF16, tag="exT", bufs=6)
                        nc.scalar.activation(ext[:], sct[:], AF.Exp, bias=0.0, scale=EXPS)
                        exts.append(ext)
                    pavt = ps.tile([128, 512], F32, tag="pav", bufs=3, name="pavt")
                    for s in range(2):
                        hh = 2 * p + s
                        for c in range(3):
                            nc.tensor.matmul(pavt[0:65, s * 128:(s + 1) * 128],
                                             vna[qb + c][:, hh * 65:hh * 65 + 65],
                                             exts[s][:, c * 128:(c + 1) * 128],
                                             start=(c == 0), stop=(c == 2))
                    # softmax 1/sum for both subs in one op (sum rows adjacent)
                    rs = attn.tile([1, 256], BF16, tag="rs", bufs=8)
                    with nc.allow_low_precision(reason="softmax 1/sum in bf16"):
                        nc.vector.reciprocal(rs[:], pavt[64:65, 0:256])
                    finalize_pair(p, qb, pavt, rs)

                # E/F one block behind D so its matmuls never stall the PE
                # queue on D(qb)'s normalization tail
                if qb >= 1:
                    emit_ef(qb - 1)
            emit_ef(3)

            # ---- Phase G: FFN (bf16) ----
            for m in range(16):
                pg = ps.tile([128, 512], F32, tag="pav", bufs=3, name="pg")
                for c in range(4):
                    nc.tensor.matmul(pg[:], w1s[c][:, :, m * 128:(m + 1) * 128],
                                     h2Tp[c][:, :, :],
                                     start=(c == 0), stop=(c == 3), perf_mode=DR)
                # gelu(pg / 64): undo the fp8 weight scale exactly
                nc.scalar.activation(gT[m][:], pg[:], AF.Gelu, scale=1.0 / WS)

            w2s = [wload(D, w2[k * 128:(k + 1) * 128, :], BF16) for k in range(16)]
            for t in range(4):
                ot = work.tile([128, D], F32, tag="ot", bufs=2)
                for nh in range(2):
                    po = ps.tile([128, 512], F32, tag="pa", bufs=2, name="po2")
                    for k in range(16):
                        nc.tensor.matmul(po[:], gT[k][:, t * 128:(t + 1) * 128],
                                         w2s[k][:, nh * 512:(nh + 1) * 512],
                                         start=(k == 0), stop=(k == 15))
                    nc.vector.tensor_tensor(out=ot[:, nh * 512:(nh + 1) * 512],
                                            in0=po[:], in1=x2[t][:, nh * 512:(nh + 1) * 512],
                                            op=ALU.add)
                nc.sync.dma_start(out_d[t * 128:(t + 1) * 128, :], ot[:])

    _CACHED["nc"] = nc
    return nc


# ---------------------------------------------------------------------------
# host wrapper
# ---------------------------------------------------------------------------
def _pair8(w, scale):
    """[K, N] f32 -> [K//256, 128, 2*N] e4m3 DoubleRow pair layout."""
    f8 = ml_dtypes.float8_e4m3
    K, N = w.shape
    w8 = (np.asarray(w, np.float32) * scale).astype(f8)
    return np.ascontiguousarray(
        w8.reshape(K // 256, 2, 128, N).transpose(0, 2, 1, 3).reshape(K // 256, 128, 2 * N))


def _host_inputs(x, qkv_w, out_w, ffn_w1, ffn_w2):
    bf = ml_dtypes.bfloat16
    shared = {
        "wq8": _pair8(qkv_w, WS),
        "wo8": _pair8(out_w, WS),
        "w18": _pair8(ffn_w1, WS),
        "w2": np.ascontiguousarray(np.asarray(ffn_w2).astype(bf)),
        "ident": np.eye(128, dtype=bf),
    }
    r = np.arange(128)
    # transposed-score masks [key_local, query]: for query i, keys j in
    # [i, i+256] of the 384-band are valid
    t_lo = np.where(r[:, None] >= r[None, :], 0.0, NEG).astype(np.float32)
    t_hi = np.where(r[:, None] <= r[None, :], 0.0, NEG).astype(np.float32)
    zeros = np.zeros((128, 128), np.float32)
    full = np.full((128, 128), NEG, np.float32)

    def band(c0, c2):
        return np.concatenate([c0, zeros, c2], axis=1)

    in_maps = []
    for core in range(8):
        b, ck = core // 4, core % 4
        lo = ck * 512 - HALO
        xsl = np.zeros((R, D), np.float32)
        s, e = max(lo, 0), min(lo + R, L)
        xsl[s - lo:e - lo] = x[b, s:e]
        m_first = band(full if ck == 0 else t_lo, t_hi)
        m_mid = band(t_lo, t_hi)
        m_last = band(t_lo, full if ck == 3 else t_hi)
        in_maps.append({
            "xs": xsl,
            "maskd": np.stack([m_first, m_mid, m_last]).astype(bf),
            **shared,
        })
    return in_maps


def kernel(x, qkv_w, qkv_b, out_w, out_b, ln1_g, ln1_b, ln2_g, ln2_b,
           ffn_w1, ffn_b1, ffn_w2, ffn_b2, _return_results=False):
    x = np.asarray(x, np.float32)
    nc = _build_program()
    in_maps = _host_inputs(x, np.asarray(qkv_w), np.asarray(out_w),
                           np.asarray(ffn_w1), np.asarray(ffn_w2))
    res = run_bass_kernel_spmd(nc, in_maps, list(range(8)))
    out = np.empty((B, L, D), np.float32)
    for core in range(8):
        b, ck = core // 4, core % 4
        out[b, ck * 512:(ck + 1) * 512] = res.results[core]["out"]
    if _return_results:
        return out, res
    return out


# revision 62
# speedup vs baseline: 1.0486x; 1.0051x over previous
"""Windowed-attention transformer layer on 8 trn2 NeuronCores.

Sharding: the 4096 (B=2 x L=2048) token rows are split into 8 contiguous
chunks of 512 (4 per batch element). Each core gets its chunk plus a
128-token halo per side (window 256), zero-padded at batch edges, and
recomputes LN1+QKV on the halo -> fully independent cores, no collectives.

Structure:
- QKV GEMM and the attention-output projection run in FP8 (e4m3) with
  DoubleRow perf mode (2 contraction rows per PE cell -> 2x matmul
  throughput).  Weights are scaled by 64 on the host to clear the e4m3
  denormal range; the scale is divided back out through the softmax
  normalization (ones column = 8 -> avT holds 8x values in fp8) and a
  1/512 factor fused into the residual add.  FFN stays bf16 (fp8 there
  would eat most of the 2e-2 error budget).
- v is computed in NATURAL layout straight from the QKV GEMM, with a
  per-head ones column appended so the attention AV matmul also produces
  the softmax denominator (augmented-V trick).
- attention scores are computed TRANSPOSED (keys on partitions) so the
  exp output feeds the AV matmul directly -- no PSUM->SBUF bounce of the
  attention weights.
- the banded window mask is ADDED ON THE PE as an accumulating matmul
  (identity stationary x mask moving) per 128-key chunk.  One PSUM bank
  sustains only one live accumulation group, so each chunk's QK+mask
  group is closed before the next chunk starts.
- softmax normalization: reciprocal of the matmul-produced sums, a K=1
  ones-matmul broadcast into spare columns of the same PSUM bank, and a
  multiply fused into the PSUM->SBUF copy.
- QKV GEMM and attention are interleaved per head-pair so PE/Act/DVE all
  stay busy; LN statistics, softmax sums, residuals are fp32.

LN gains/biases and linear biases are identities per the input spec and
are skipped.
"""

import numpy as np
import ml_dtypes

import concourse.bass as bass
import concourse.tile as tile
from concourse import mybir
from concourse.bass_utils import run_bass_kernel_spmd
from concourse.vector_clock import ScopedClock, VectorClock
from concourse.tile_scheduler import N_PROCS

F32 = mybir.dt.float32
BF16 = mybir.dt.bfloat16
F8 = mybir.dt.float8e4
AF = mybir.ActivationFunctionType
ALU = mybir.AluOpType
DR = mybir.MatmulPerfMode.DoubleRow

B, L, D = 2, 2048, 1024
H, HD = 16, 64
R = 768          # local rows incl. halo
OWN = 512        # owned rows per core
HALO = 128
NEG = -1.0e9
WS = 64.0        # host-side fp8 weight scale for wq/wo
ONEC = 8.0       # vna ones column: makes avT = 8 * av (fp8 range), 64/8=8
EXPS = 0.125 / (WS * WS)   # exp scale absorbs q,k both carrying x64


class SplitWaitTileContext(tile.TileContext):
    """Walrus in this container allows at most ONE sync wait per
    instruction: split extra waits onto preceding same-engine NoOps, and
    emit the tail drain as one drain per outstanding proc."""
    _ctr = 0

    def _add_instruction(self, inst):
        si = inst.sync_info
        if si is not None and si.on_wait and len(si.on_wait) > 1:
            waits = list(si.on_wait)
            for w in waits[:-1]:
                SplitWaitTileContext._ctr += 1
                nop = mybir.InstNoOp(name=f"splitw-{SplitWaitTileContext._ctr}", ins=[], outs=[])
                nop.engine = inst.engine
                nop.sync_info = mybir.SyncInfo(on_wait=[w], on_update=[])
                super()._add_instruction(nop)
            inst.sync_info = mybir.SyncInfo(on_wait=[waits[-1]], on_update=list(si.on_update))
        super()._add_instruction(inst)

    def _drain_and_barrier(self, tick_clock, wait_clock):
        gc = tick_clock.global_clock
        for p in range(N_PROCS):
            if gc[p] > 0:
                vals = [0] * N_PROCS
                vals[p] = gc[p]
                d = self.nc.sync.drain()
                wait_clock.add_sem_waits(d.ins, ScopedClock({None: VectorClock(vals)}))
        self.nc.sync.drain()
        self.nc.all_engine_barrier()
        assert self.sems is not None
        popped = self.nc._tile_sem_poison_stack.pop()
        assert popped is self._sem_poison
        self.nc.clear_and_free_semaphores(list(self.sems.allocated().values()))
        self.nc.all_engine_barrier()


# ---------------------------------------------------------------------------
# device program (identical on all 8 cores; only input data differs)
# ---------------------------------------------------------------------------
_CACHED = {}


def _build_program():
    if "nc" in _CACHED:
        return _CACHED["nc"]

    nc = bass.Bass("TRN2", target_bir_lowering=False, debug=False, num_devices=1)

    xs = nc.dram_tensor("xs", [R, D], F32, kind="ExternalInput").ap()
    # fp8 DoubleRow pair layouts: [pair, 128, 2*cols]
    wq8 = nc.dram_tensor("wq8", [4, 128, 2 * 3 * D], F8, kind="ExternalInput").ap()
    wo8 = nc.dram_tensor("wo8", [4, 128, 2 * D], F8, kind="ExternalInput").ap()
    w18 = nc.dram_tensor("w18", [4, 128, 2 * 2 * D], F8, kind="ExternalInput").ap()
    w2 = nc.dram_tensor("w2", [2 * D, D], BF16, kind="ExternalInput").ap()
    ident_d = nc.dram_tensor("ident", [128, 128], BF16, kind="ExternalInput").ap()
    mask_d = nc.dram_tensor("maskd", [3, 128, 384], BF16, kind="ExternalInput").ap()
    out_d = nc.dram_tensor("out", [OWN, D], F32, kind="ExternalOutput").ap()

    cp = [0]  # copy engine round-robin

    def copy(dst, src):
        cp[0] ^= 1
        if cp[0]:
            nc.vector.tensor_copy(dst, src)
        else:
            nc.scalar.copy(dst, src)

    with SplitWaitTileContext(nc) as tc:
        with (
            tc.tile_pool(name="per", bufs=1) as per,      # persistent
            tc.tile_pool(name="xq", bufs=6) as xq,        # x tiles (fp32)
            tc.tile_pool(name="work", bufs=3) as work,    # h tiles / out tiles
            tc.tile_pool(name="attn", bufs=6) as attn,    # small LN/attention tiles
            tc.tile_pool(name="wts", bufs=16) as wts,     # streamed weights 2KB class
            tc.tile_pool(name="w1p", bufs=4) as w1p,      # ffn_w1 chunks 4KB class
            tc.tile_pool(name="ps", bufs=1, space="PSUM") as ps,
        ):
            # x tiles first on the SP queue so phase A starts ASAP
            xts = []
            for t in range(6):
                xt = xq.tile([128, D], F32, tag="xt", name=f"xpre{t}")
                nc.sync.dma_start(xt[:], xs[t * 128:(t + 1) * 128, :])
                xts.append(xt)
            ident = per.tile([128, 128], BF16, tag="ident")
            nc.gpsimd.dma_start(ident[:], ident_d[:])
            masks = []
            for i in range(3):
                m = per.tile([128, 384], BF16, tag=f"mask{i}")
                nc.gpsimd.dma_start(m[:], mask_d[i])
                masks.append(m)
            mask_for_qb = [masks[0], masks[1], masks[1], masks[2]]

            epsb = per.tile([128, 1], F32, tag="epsb")
            nc.vector.memset(epsb[:], 1e-5)
            ones64 = per.tile([1, 64], BF16, tag="ones64")
            nc.vector.memset(ones64[:], 1.0)


            # persistent activations
            hTp = [per.tile([128, 2, R], F8, tag=f"hTp{c}", name=f"hTp{c}") for c in range(4)]
            qT = [per.tile([128, OWN], BF16, tag=f"qT{d}", name=f"qT{d}") for d in range(8)]
            kT = [per.tile([128, R], BF16, tag=f"kT{d}", name=f"kT{d}") for d in range(8)]
            vna = [per.tile([128, 1040], BF16, tag=f"vna{t}", name=f"vna{t}") for t in range(6)]
            avTp = [per.tile([128, 2, OWN], F8, tag=f"avTp{c}", name=f"avTp{c}") for c in range(4)]
            x2 = [per.tile([128, D], F32, tag=f"x2_{t}", name=f"x2_{t}") for t in range(4)]
            h2Tp = [per.tile([128, 2, OWN], F8, tag=f"h2Tp{c}", name=f"h2Tp{c}") for c in range(4)]
            gT = [per.tile([128, OWN], BF16, tag=f"gT{m}", name=f"gT{m}") for m in range(16)]

            # weight loads on the SP queue (after the x tiles above)
            def wload(cols, src, dt=F8):
                t = wts.tile([128, cols], dt, tag="wchunk", name="wt")
                nc.sync.dma_start(t[:], src)
                return t

            def wsec(sec):
                # pair tiles [128, 2, 1024] of wq8 section sec (q=0, k=1, v=2)
                out = []
                for c in range(4):
                    w = wq8[c].rearrange("p (i n) -> p i n", i=2)[:, :, sec * D:(sec + 1) * D]
                    t = wts.tile([128, 2, D], F8, tag="wchunk", name="wt")
                    nc.sync.dma_start(t[:], w)
                    out.append(t)
                return out

            wv = wsec(2)
            wqs = wsec(0)

            # ---- Phase A: LN1 + transpose -> hTp (fp8) ----
            def layernorm_tile(xt, h):
                st = attn.tile([128, 12], F32, tag="st")
                nc.vector.bn_stats(st[:, 0:6], xt[:, 0:512])
                nc.vector.bn_stats(st[:, 6:12], xt[:, 512:1024])
                mv = attn.tile([128, 2], F32, tag="mv")
                nc.vector.bn_aggr(mv[:], st[:].rearrange("p (g s) -> p g s", g=2))
                std = attn.tile([128, 1], F32, tag="std")
                nc.scalar.activation(std[:], mv[:, 1:2], AF.Sqrt, bias=epsb[:])
                rstd = attn.tile([128, 1], F32, tag="rstd")
                nc.vector.reciprocal(rstd[:], std[:])
                negmu = attn.tile([128, 1], F32, tag="negmu")
                nc.vector.tensor_scalar(out=negmu[:], in0=mv[:, 0:1], scalar1=-1.0,
                                        scalar2=None, op0=ALU.mult)
                neg = attn.tile([128, 1], F32, tag="neg")
                nc.vector.tensor_scalar(out=neg[:], in0=negmu[:], scalar1=rstd[:],
                                        scalar2=None, op0=ALU.mult)
                nc.scalar.activation(h[:], xt[:], AF.Identity, bias=neg[:], scale=rstd[:])

            for t in range(6):
                xt = xts[t]
                h = work.tile([128, D], BF16, tag="h")
                layernorm_tile(xt, h)
                for d in range(8):
                    pt = ps.tile([128, 128], BF16, tag="sc", bufs=3, name="pt")
                    nc.tensor.transpose(pt[:], h[:, d * 128:(d + 1) * 128], ident[:])
                    copy(hTp[d // 2][:, d % 2, t * 128:(t + 1) * 128], pt[:])
                # ---- Phase B-v for this tile: v natural with ones column ----
                # (values carry x64 from the weight scale; ones = 8 so the
                #  softmax normalization leaves avT = 8 * av for fp8 range)
                ones_cols = vna[t].rearrange("p (h x) -> p h x", x=65)[:, :, 64:65]
                nc.vector.memset(ones_cols, ONEC)
                for nh in range(2):
                    pv = ps.tile([128, 512], F32, tag="pa", bufs=2, name="pv")
                    for c in range(4):
                        nc.tensor.matmul(pv[:], hTp[c][:, :, t * 128:(t + 1) * 128],
                                         wv[c][:, :, nh * 512:(nh + 1) * 512],
                                         start=(c == 0), stop=(c == 3), perf_mode=DR)
                    dst = vna[t].rearrange("p (h x) -> p h x", x=65)[:, nh * 8:(nh + 1) * 8, 0:64]
                    copy(dst, pv[:].rearrange("p (h d) -> p h d", d=64))

            wk = wsec(1)

            # ---- Phase B q/k (all head-pairs) ----
            for p in range(8):
                # q m-tile p: heads 2p, 2p+1; own tokens only
                pq = ps.tile([128, 512], F32, tag="sc", bufs=3, name="pq")
                for c in range(4):
                    nc.tensor.matmul(pq[:], wqs[c][:, :, p * 128:(p + 1) * 128],
                                     hTp[c][:, :, HALO:HALO + OWN],
                                     start=(c == 0), stop=(c == 3), perf_mode=DR)
                copy(qT[p][:], pq[:])
                # k m-tile p: all R halo tokens, two half-passes
                for half in range(2):
                    pk = ps.tile([128, 384], F32, tag="sc", bufs=3, name="pk")
                    for c in range(4):
                        nc.tensor.matmul(pk[:], wk[c][:, :, p * 128:(p + 1) * 128],
                                         hTp[c][:, :, half * 384:(half + 1) * 384],
                                         start=(c == 0), stop=(c == 3), perf_mode=DR)
                    copy(kT[p][:, half * 384:(half + 1) * 384], pk[:])

            # prefetch out-proj / ffn1 weights while attention runs
            wos = []
            for c in range(4):
                wt = wts.tile([128, 2, D], F8, tag="wchunk", name="wt")
                nc.sync.dma_start(wt[:], wo8[c].rearrange("p (i n) -> p i n", i=2))
                wos.append(wt)
            w1s = []
            for c in range(4):
                wt = w1p.tile([128, 2, 2 * D], F8, tag="w1c", name="wt")
                nc.sync.dma_start(wt[:], w18[c].rearrange("p (i n) -> p i n", i=2))
                w1s.append(wt)

            # ---- Phase D attention (query-block outer), E/F interleaved:
            #      after query block qb completes for all heads, that token
            #      tile's out-proj + residual + LN2 runs in D's engine gaps
            def emit_ef(t):
                # out-proj + residual + LN2 + transpose for token tile t
                # residual rows are x tiles 1..4 from phase A, still resident
                xo = xts[t + 1]
                for nh in range(2):
                    po = ps.tile([128, 512], F32, tag="pa", bufs=2, name="po")
                    for c in range(4):
                        nc.tensor.matmul(po[:], avTp[c][:, :, t * 128:(t + 1) * 128],
                                         wos[c][:, :, nh * 512:(nh + 1) * 512],
                                         start=(c == 0), stop=(c == 3), perf_mode=DR)
                    # x2 = po / (8 * 64) + x   (avT carries x8, wo carries x64)
                    nc.vector.scalar_tensor_tensor(
                        out=x2[t][:, nh * 512:(nh + 1) * 512], in0=po[:],
                        scalar=1.0 / (ONEC * WS), in1=xo[:, nh * 512:(nh + 1) * 512],
                        op0=ALU.mult, op1=ALU.add)
                h2 = work.tile([128, D], BF16, tag="h2")
                layernorm_tile(x2[t], h2)
                for d in range(8):
                    pt = ps.tile([128, 128], BF16, tag="sc", bufs=3, name="pt2")
                    nc.tensor.transpose(pt[:], h2[:, d * 128:(d + 1) * 128], ident[:])
                    copy(h2Tp[d // 2][:, d % 2, t * 128:(t + 1) * 128], pt[:])

            def finalize_pair(p, qb, pavt, rs):
                # deferred softmax-normalize of pair (qb, p): runs one pair
                # behind so the PE bcast never stalls on the DVE reciprocal
                for s in range(2):
                    nc.tensor.matmul(pavt[0:64, 256 + s * 128:256 + (s + 1) * 128],
                                     ones64[:], rs[:, s * 128:(s + 1) * 128],
                                     start=True, stop=True)
                    rsb = attn.tile([64, 128], BF16, tag="rsb", bufs=4)
                    copy(rsb[:], pavt[0:64, 256 + s * 128:256 + (s + 1) * 128])
                    nc.vector.tensor_tensor(
                        out=avTp[p // 2][s * 64:(s + 1) * 64, p % 2,
                                         qb * 128:(qb + 1) * 128],
                        in0=pavt[0:64, s * 128:(s + 1) * 128], in1=rsb[:],
                        op=ALU.mult)

            for qb in range(4):
                for p in range(8):
                    exts = []
                    for s in range(2):
                        # one PSUM bank supports a single live accumulation
                        # group on HW: close each chunk's group (QK + mask
                        # add) before the next chunk's start re-arms the bank
                        sct = ps.tile([128, 384], F32, tag="sc", bufs=3, name="sct")
                        for c in range(3):
                            kc = kT[p][s * 64:s * 64 + 64,
                                       qb * 128 + c * 128:qb * 128 + (c + 1) * 128]
                            qs = qT[p][s * 64:s * 64 + 64, qb * 128:(qb + 1) * 128]
                            nc.tensor.matmul(sct[:, c * 128:(c + 1) * 128], kc, qs,
                                             start=True, stop=False)
                            nc.tensor.matmul(sct[:, c * 128:(c + 1) * 128], ident[:],
                                             mask_for_qb[qb][:, c * 128:(c + 1) * 128],
                                             start=False, stop=True)
                        ext = attn.tile([128, 384], BF16, tag="exT", bufs=6)
                        nc.scalar.activation(ext[:], sct[:], AF.Exp, bias=0.0, scale=EXPS)
                        exts.append(ext)
                    pavt = ps.tile([128, 512], F32, tag="pav", bufs=3, name="pavt")
                    for s in range(2):
                        hh = 2 * p + s
                        for c in range(3):
                            nc.tensor.matmul(pavt[0:65, s * 128:(s + 1) * 128],
                                             vna[qb + c][:, hh * 65:hh * 65 + 65],
                                             exts[s][:, c * 128:(c + 1) * 128],
                                             start=(c == 0), stop=(c == 2))
                    # softmax 1/sum for both subs in one op (sum rows adjacent)
                    rs = attn.tile([1, 256], BF16, tag="rs", bufs=8)
                    with nc.allow_low_precision(reason="softmax 1/sum in bf16"):
                        nc.vector.reciprocal(rs[:], pavt[64:65, 0:256])
                    finalize_pair(p, qb, pavt, rs)

                # E/F one block behind D so its matmuls never stall the PE
                # queue on D(qb)'s normalization tail
                if qb >= 1:
                    emit_ef(qb - 1)
            emit_ef(3)

            # ---- Phase G: FFN (bf16) ----
            for m in range(16):
                pg = ps.tile([128, 512], F32, tag="pav", bufs=3, name="pg")
                for c in range(4):
                    nc.tensor.matmul(pg[:], w1s[c][:, :, m * 128:(m + 1) * 128],
                                     h2Tp[c][:, :, :],
                                     start=(c == 0), stop=(c == 3), perf_mode=DR)
                # gelu(pg / 64): undo the fp8 weight scale exactly
                nc.scalar.activation(gT[m][:], pg[:], AF.Gelu, scale=1.0 / WS)

            w2s = [wload(D, w2[k * 128:(k + 1) * 128, :], BF16) for k in range(16)]
            for t in range(4):
                ot = work.tile([128, D], F32, tag="ot", bufs=2)
                for nh in range(2):
                    po = ps.tile([128, 512], F32, tag="pa", bufs=2, name="po2")
                    for k in range(16):
                        nc.tensor.matmul(po[:], gT[k][:, t * 128:(t + 1) * 128],
                                         w2s[k][:, nh * 512:(nh + 1) * 512],
                                         start=(k == 0), stop=(k == 15))
                    nc.vector.tensor_tensor(out=ot[:, nh * 512:(nh + 1) * 512],
                                            in0=po[:], in1=x2[t][:, nh * 512:(nh + 1) * 512],
                                            op=ALU.add)
                    nc.sync.dma_start(out_d[t * 128:(t + 1) * 128, nh * 512:(nh + 1) * 512],
                                      ot[:, nh * 512:(nh + 1) * 512])

    _CACHED["nc"] = nc
    return nc


# ---------------------------------------------------------------------------
# host wrapper
# ---------------------------------------------------------------------------
def _pair8(w, scale):
    """[K, N] f32 -> [K//256, 128, 2*N] e4m3 DoubleRow pair layout."""
    f8 = ml_dtypes.float8_e4m3
    K, N = w.shape
    w8 = (np.asarray(w, np.float32) * scale).astype(f8)
    return np.ascontiguousarray(
        w8.reshape(K // 256, 2, 128, N).transpose(0, 2, 1, 3).reshape(K // 256, 128, 2 * N))


def _host_inputs(x, qkv_w, out_w, ffn_w1, ffn_w2):
    bf = ml_dtypes.bfloat16
    shared = {
        "wq8": _pair8(qkv_w, WS),
        "wo8": _pair8(out_w, WS),
        "w18": _pair8(ffn_w1, WS),
        "w2": np.ascontiguousarray(np.asarray(ffn_w2).astype(bf)),
        "ident": np.eye(128, dtype=bf),
    }
    r = np.arange(128)
    # transposed-score masks [key_local, query]: for query i, keys j in
    # [i, i+256] of the 384-band are valid
    t_lo = np.where(r[:, None] >= r[None, :], 0.0, NEG).astype(np.float32)
    t_hi = np.where(r[:, None] <= r[None, :], 0.0, NEG).astype(np.float32)
    zeros = np.zeros((128, 128), np.float32)
    full = np.full((128, 128), NEG, np.float32)

    def band(c0, c2):
        return np.concatenate([c0, zeros, c2], axis=1)

    in_maps = []
    for core in range(8):
        b, ck = core // 4, core % 4
        lo = ck * 512 - HALO
        xsl = np.zeros((R, D), np.float32)
        s, e = max(lo, 0), min(lo + R, L)
        xsl[s - lo:e - lo] = x[b, s:e]
        m_first = band(full if ck == 0 else t_lo, t_hi)
        m_mid = band(t_lo, t_hi)
        m_last = band(t_lo, full if ck == 3 else t_hi)
        in_maps.append({
            "xs": xsl,
            "maskd": np.stack([m_first, m_mid, m_last]).astype(bf),
            **shared,
        })
    return in_maps


def kernel(x, qkv_w, qkv_b, out_w, out_b, ln1_g, ln1_b, ln2_g, ln2_b,
           ffn_w1, ffn_b1, ffn_w2, ffn_b2, _return_results=False):
    x = np.asarray(x, np.float32)
    nc = _build_program()
    in_maps = _host_inputs(x, np.asarray(qkv_w), np.asarray(out_w),
                           np.asarray(ffn_w1), np.asarray(ffn_w2))
    res = run_bass_kernel_spmd(nc, in_maps, list(range(8)))
    out = np.empty((B, L, D), np.float32)
    for core in range(8):
        b, ck = core // 4, core % 4
        out[b, ck * 512:(ck + 1) * 512] = res.results[core]["out"]
    if _return_results:
        return out, res
    return out


# revision 66
# speedup vs baseline: 1.0491x; 1.0005x over previous
"""Windowed-attention transformer layer on 8 trn2 NeuronCores.

Sharding: the 4096 (B=2 x L=2048) token rows are split into 8 contiguous
chunks of 512 (4 per batch element). Each core gets its chunk plus a
128-token halo per side (window 256), zero-padded at batch edges, and
recomputes LN1+QKV on the halo -> fully independent cores, no collectives.

Structure:
- QKV GEMM and the attention-output projection run in FP8 (e4m3) with
  DoubleRow perf mode (2 contraction rows per PE cell -> 2x matmul
  throughput).  Weights are scaled by 64 on the host to clear the e4m3
  denormal range; the scale is divided back out through the softmax
  normalization (ones column = 8 -> avT holds 8x values in fp8) and a
  1/512 factor fused into the residual add.  FFN stays bf16 (fp8 there
  would eat most of the 2e-2 error budget).
- v is computed in NATURAL layout straight from the QKV GEMM, with a
  per-head ones column appended so the attention AV matmul also produces
  the softmax denominator (augmented-V trick).
- attention scores are computed TRANSPOSED (keys on partitions) so the
  exp output feeds the AV matmul directly -- no PSUM->SBUF bounce of the
  attention weights.
- the banded window mask is ADDED ON THE PE as an accumulating matmul
  (identity stationary x mask moving) per 128-key chunk.  One PSUM bank
  sustains only one live accumulation group, so each chunk's QK+mask
  group is closed before the next chunk starts.
- softmax normalization: reciprocal of the matmul-produced sums, a K=1
  ones-matmul broadcast into spare columns of the same PSUM bank, and a
  multiply fused into the PSUM->SBUF copy.
- QKV GEMM and attention are interleaved per head-pair so PE/Act/DVE all
  stay busy; LN statistics, softmax sums, residuals are fp32.

LN gains/biases and linear biases are identities per the input spec and
are skipped.
"""

import numpy as np
import ml_dtypes

import concourse.bass as bass
import concourse.tile as tile
from concourse import mybir
from concourse.bass_utils import run_bass_kernel_spmd
from concourse.vector_clock import ScopedClock, VectorClock
from concourse.tile_scheduler import N_PROCS

F32 = mybir.dt.float32
BF16 = mybir.dt.bfloat16
F8 = mybir.dt.float8e4
AF = mybir.ActivationFunctionType
ALU = mybir.AluOpType
DR = mybir.MatmulPerfMode.DoubleRow

B, L, D = 2, 2048, 1024
H, HD = 16, 64
R = 768          # local rows incl. halo
OWN = 512        # owned rows per core
HALO = 128
NEG = -1.0e9
WS = 64.0        # host-side fp8 weight scale for wq/wo
ONEC = 8.0       # vna ones column: makes avT = 8 * av (fp8 range), 64/8=8
EXPS = 0.125 / (WS * WS)   # exp scale absorbs q,k both carrying x64


class SplitWaitTileContext(tile.TileContext):
    """Walrus in this container allows at most ONE sync wait per
    instruction: split extra waits onto preceding same-engine NoOps, and
    emit the tail drain as one drain per outstanding proc."""
    _ctr = 0

    def _add_instruction(self, inst):
        si = inst.sync_info
        if si is not None and si.on_wait and len(si.on_wait) > 1:
            waits = list(si.on_wait)
            for w in waits[:-1]:
                SplitWaitTileContext._ctr += 1
                nop = mybir.InstNoOp(name=f"splitw-{SplitWaitTileContext._ctr}", ins=[], outs=[])
                nop.engine = inst.engine
                nop.sync_info = mybir.SyncInfo(on_wait=[w], on_update=[])
                super()._add_instruction(nop)
            inst.sync_info = mybir.SyncInfo(on_wait=[waits[-1]], on_update=list(si.on_update))
        super()._add_instruction(inst)

    def _drain_and_barrier(self, tick_clock, wait_clock):
        gc = tick_clock.global_clock
        for p in range(N_PROCS):
            if gc[p] > 0:
                vals = [0] * N_PROCS
                vals[p] = gc[p]
                d = self.nc.sync.drain()
                wait_clock.add_sem_waits(d.ins, ScopedClock({None: VectorClock(vals)}))
        self.nc.sync.drain()
        self.nc.all_engine_barrier()
        assert self.sems is not None
        popped = self.nc._tile_sem_poison_stack.pop()
        assert popped is self._sem_poison
        self.nc.clear_and_free_semaphores(list(self.sems.allocated().values()))
        self.nc.all_engine_barrier()


# ---------------------------------------------------------------------------
# device program (identical on all 8 cores; only input data differs)
# ---------------------------------------------------------------------------
_CACHED = {}


def _build_program():
    if "nc" in _CACHED:
        return _CACHED["nc"]

    nc = bass.Bass("TRN2", target_bir_lowering=False, debug=False, num_devices=1)

    xs = nc.dram_tensor("xs", [R, D], F32, kind="ExternalInput").ap()
    # fp8 DoubleRow pair layouts: [pair, 128, 2*cols]
    wq8 = nc.dram_tensor("wq8", [4, 128, 2 * 3 * D], F8, kind="ExternalInput").ap()
    wo8 = nc.dram_tensor("wo8", [4, 128, 2 * D], F8, kind="ExternalInput").ap()
    w18 = nc.dram_tensor("w18", [4, 128, 2 * 2 * D], F8, kind="ExternalInput").ap()
    w2 = nc.dram_tensor("w2", [2 * D, D], BF16, kind="ExternalInput").ap()
    ident_d = nc.dram_tensor("ident", [128, 128], BF16, kind="ExternalInput").ap()
    mask_d = nc.dram_tensor("maskd", [3, 128, 384], BF16, kind="ExternalInput").ap()
    out_d = nc.dram_tensor("out", [OWN, D], F32, kind="ExternalOutput").ap()

    cp = [0]  # copy engine round-robin

    def copy(dst, src):
        cp[0] ^= 1
        if cp[0]:
            nc.vector.tensor_copy(dst, src)
        else:
            nc.scalar.copy(dst, src)

    with SplitWaitTileContext(nc) as tc:
        with (
            tc.tile_pool(name="per", bufs=1) as per,      # persistent
            tc.tile_pool(name="xq", bufs=6) as xq,        # x tiles (fp32)
            tc.tile_pool(name="work", bufs=2) as work,    # h tiles / out tiles
            tc.tile_pool(name="attn", bufs=6) as attn,    # small LN/attention tiles
            tc.tile_pool(name="wts", bufs=16) as wts,     # streamed weights 2KB class
            tc.tile_pool(name="w1p", bufs=4) as w1p,      # ffn_w1 chunks 4KB class
            tc.tile_pool(name="ps", bufs=1, space="PSUM") as ps,
        ):
            # x tiles first on the SP queue so phase A starts ASAP
            xts = []
            for t in range(6):
                xt = xq.tile([128, D], F32, tag="xt", name=f"xpre{t}")
                nc.sync.dma_start(xt[:], xs[t * 128:(t + 1) * 128, :])
                xts.append(xt)
            ident = per.tile([128, 128], BF16, tag="ident")
            nc.gpsimd.dma_start(ident[:], ident_d[:])
            masks = []
            for i in range(3):
                m = per.tile([128, 384], BF16, tag=f"mask{i}")
                nc.gpsimd.dma_start(m[:], mask_d[i])
                masks.append(m)
            mask_for_qb = [masks[0], masks[1], masks[1], masks[2]]

            epsb = per.tile([128, 1], F32, tag="epsb")
            nc.vector.memset(epsb[:], 1e-5)
            ones64 = per.tile([1, 64], BF16, tag="ones64")
            nc.vector.memset(ones64[:], 1.0)


            # persistent activations
            hTp = [per.tile([128, 2, R], F8, tag=f"hTp{c}", name=f"hTp{c}") for c in range(4)]
            qT = [per.tile([128, OWN], BF16, tag=f"qT{d}", name=f"qT{d}") for d in range(8)]
            kT = [per.tile([128, R], BF16, tag=f"kT{d}", name=f"kT{d}") for d in range(8)]
            vna = [per.tile([128, 1040], BF16, tag=f"vna{t}", name=f"vna{t}") for t in range(6)]
            avTp = [per.tile([128, 2, OWN], F8, tag=f"avTp{c}", name=f"avTp{c}") for c in range(4)]
            x2 = [per.tile([128, D], F32, tag=f"x2_{t}", name=f"x2_{t}") for t in range(4)]
            h2Tp = [per.tile([128, 2, OWN], F8, tag=f"h2Tp{c}", name=f"h2Tp{c}") for c in range(4)]
            gT = [per.tile([128, OWN], BF16, tag=f"gT{m}", name=f"gT{m}") for m in range(16)]

            # weight loads on the SP queue (after the x tiles above)
            def wload(cols, src, dt=F8):
                t = wts.tile([128, cols], dt, tag="wchunk", name="wt")
                nc.sync.dma_start(t[:], src)
                return t

            def wsec(sec):
                # pair tiles [128, 2, 1024] of wq8 section sec (q=0, k=1, v=2)
                out = []
                for c in range(4):
                    w = wq8[c].rearrange("p (i n) -> p i n", i=2)[:, :, sec * D:(sec + 1) * D]
                    t = wts.tile([128, 2, D], F8, tag="wchunk", name="wt")
                    nc.sync.dma_start(t[:], w)
                    out.append(t)
                return out

            wv = wsec(2)
            wqs = wsec(0)

            # ---- Phase A: LN1 + transpose -> hTp (fp8) ----
            def layernorm_tile(xt, h):
                st = attn.tile([128, 12], F32, tag="st")
                nc.vector.bn_stats(st[:, 0:6], xt[:, 0:512])
                nc.vector.bn_stats(st[:, 6:12], xt[:, 512:1024])
                mv = attn.tile([128, 2], F32, tag="mv")
                nc.vector.bn_aggr(mv[:], st[:].rearrange("p (g s) -> p g s", g=2))
                std = attn.tile([128, 1], F32, tag="std")
                nc.scalar.activation(std[:], mv[:, 1:2], AF.Sqrt, bias=epsb[:])
                rstd = attn.tile([128, 1], F32, tag="rstd")
                nc.vector.reciprocal(rstd[:], std[:])
                negmu = attn.tile([128, 1], F32, tag="negmu")
                nc.vector.tensor_scalar(out=negmu[:], in0=mv[:, 0:1], scalar1=-1.0,
                                        scalar2=None, op0=ALU.mult)
                neg = attn.tile([128, 1], F32, tag="neg")
                nc.vector.tensor_scalar(out=neg[:], in0=negmu[:], scalar1=rstd[:],
                                        scalar2=None, op0=ALU.mult)
                nc.scalar.activation(h[:], xt[:], AF.Identity, bias=neg[:], scale=rstd[:])

            for t in range(6):
                xt = xts[t]
                h = work.tile([128, D], BF16, tag="h")
                layernorm_tile(xt, h)
                for d in range(8):
                    pt = ps.tile([128, 128], BF16, tag="sc", bufs=3, name="pt")
                    nc.tensor.transpose(pt[:], h[:, d * 128:(d + 1) * 128], ident[:])
                    copy(hTp[d // 2][:, d % 2, t * 128:(t + 1) * 128], pt[:])
                # ---- Phase B-v for this tile: v natural with ones column ----
                # (values carry x64 from the weight scale; ones = 8 so the
                #  softmax normalization leaves avT = 8 * av for fp8 range)
                ones_cols = vna[t].rearrange("p (h x) -> p h x", x=65)[:, :, 64:65]
                nc.vector.memset(ones_cols, ONEC)
                for nh in range(2):
                    pv = ps.tile([128, 512], F32, tag="pa", bufs=2, name="pv")
                    for c in range(4):
                        nc.tensor.matmul(pv[:], hTp[c][:, :, t * 128:(t + 1) * 128],
                                         wv[c][:, :, nh * 512:(nh + 1) * 512],
                                         start=(c == 0), stop=(c == 3), perf_mode=DR)
                    dst = vna[t].rearrange("p (h x) -> p h x", x=65)[:, nh * 8:(nh + 1) * 8, 0:64]
                    copy(dst, pv[:].rearrange("p (h d) -> p h d", d=64))

            wk = wsec(1)

            # ---- Phase B q/k (all head-pairs) ----
            for p in range(8):
                # q m-tile p: heads 2p, 2p+1; own tokens only
                pq = ps.tile([128, 512], F32, tag="sc", bufs=3, name="pq")
                for c in range(4):
                    nc.tensor.matmul(pq[:], wqs[c][:, :, p * 128:(p + 1) * 128],
                                     hTp[c][:, :, HALO:HALO + OWN],
                                     start=(c == 0), stop=(c == 3), perf_mode=DR)
                copy(qT[p][:], pq[:])
                # k m-tile p: all R halo tokens, two half-passes
                for half in range(2):
                    pk = ps.tile([128, 384], F32, tag="sc", bufs=3, name="pk")
                    for c in range(4):
                        nc.tensor.matmul(pk[:], wk[c][:, :, p * 128:(p + 1) * 128],
                                         hTp[c][:, :, half * 384:(half + 1) * 384],
                                         start=(c == 0), stop=(c == 3), perf_mode=DR)
                    copy(kT[p][:, half * 384:(half + 1) * 384], pk[:])

            # prefetch out-proj / ffn1 weights while attention runs
            wos = []
            for c in range(4):
                wt = wts.tile([128, 2, D], F8, tag="wchunk", name="wt")
                nc.sync.dma_start(wt[:], wo8[c].rearrange("p (i n) -> p i n", i=2))
                wos.append(wt)
            w1s = []
            for c in range(4):
                wt = w1p.tile([128, 2, 2 * D], F8, tag="w1c", name="wt")
                nc.sync.dma_start(wt[:], w18[c].rearrange("p (i n) -> p i n", i=2))
                w1s.append(wt)

            # ---- Phase D attention (query-block outer), E/F interleaved:
            #      after query block qb completes for all heads, that token
            #      tile's out-proj + residual + LN2 runs in D's engine gaps
            def emit_ef(t):
                # out-proj + residual + LN2 + transpose for token tile t
                # residual rows are x tiles 1..4 from phase A, still resident
                xo = xts[t + 1]
                for nh in range(2):
                    po = ps.tile([128, 512], F32, tag="pa", bufs=2, name="po")
                    for c in range(4):
                        nc.tensor.matmul(po[:], avTp[c][:, :, t * 128:(t + 1) * 128],
                                         wos[c][:, :, nh * 512:(nh + 1) * 512],
                                         start=(c == 0), stop=(c == 3), perf_mode=DR)
                    # x2 = po / (8 * 64) + x   (avT carries x8, wo carries x64)
                    nc.vector.scalar_tensor_tensor(
                        out=x2[t][:, nh * 512:(nh + 1) * 512], in0=po[:],
                        scalar=1.0 / (ONEC * WS), in1=xo[:, nh * 512:(nh + 1) * 512],
                        op0=ALU.mult, op1=ALU.add)
                h2 = work.tile([128, D], BF16, tag="h2")
                layernorm_tile(x2[t], h2)
                for d in range(8):
                    pt = ps.tile([128, 128], BF16, tag="sc", bufs=3, name="pt2")
                    nc.tensor.transpose(pt[:], h2[:, d * 128:(d + 1) * 128], ident[:])
                    copy(h2Tp[d // 2][:, d % 2, t * 128:(t + 1) * 128], pt[:])

            def finalize_pair(p, qb, pavt, rs):
                # deferred softmax-normalize of pair (qb, p): runs one pair
                # behind so the PE bcast never stalls on the DVE reciprocal
                for s in range(2):
                    nc.tensor.matmul(pavt[0:64, 256 + s * 128:256 + (s + 1) * 128],
                                     ones64[:], rs[:, s * 128:(s + 1) * 128],
                                     start=True, stop=True)
                    rsb = attn.tile([64, 128], BF16, tag="rsb", bufs=4)
                    copy(rsb[:], pavt[0:64, 256 + s * 128:256 + (s + 1) * 128])
                    nc.vector.tensor_tensor(
                        out=avTp[p // 2][s * 64:(s + 1) * 64, p % 2,
                                         qb * 128:(qb + 1) * 128],
                        in0=pavt[0:64, s * 128:(s + 1) * 128], in1=rsb[:],
                        op=ALU.mult)

            for qb in range(4):
                for p in range(8):
                    exts = []
                    for s in range(2):
                        # one PSUM bank supports a single live accumulation
                        # group on HW: close each chunk's group (QK + mask
                        # add) before the next chunk's start re-arms the bank
                        sct = ps.tile([128, 384], F32, tag="sc", bufs=3, name="sct")
                        for c in range(3):
                            kc = kT[p][s * 64:s * 64 + 64,
                                       qb * 128 + c * 128:qb * 128 + (c + 1) * 128]
                            qs = qT[p][s * 64:s * 64 + 64, qb * 128:(qb + 1) * 128]
                            nc.tensor.matmul(sct[:, c * 128:(c + 1) * 128], kc, qs,
                                             start=True, stop=False)
                            nc.tensor.matmul(sct[:, c * 128:(c + 1) * 128], ident[:],
                                             mask_for_qb[qb][:, c * 128:(c + 1) * 128],
                                             start=False, stop=True)
                        ext = attn.tile([128, 384], BF16, tag="exT", bufs=6)
                        nc.scalar.activation(ext[:], sct[:], AF.Exp, bias=0.0, scale=EXPS)
                        exts.append(ext)
                    pavt = ps.tile([128, 512], F32, tag="pav", bufs=3, name="pavt")
                    for s in range(2):
                        hh = 2 * p + s
                        for c in range(3):
                            nc.tensor.matmul(pavt[0:65, s * 128:(s + 1) * 128],
                                             vna[qb + c][:, hh * 65:hh * 65 + 65],
                                             exts[s][:, c * 128:(c + 1) * 128],
                                             start=(c == 0), stop=(c == 2))
                    # softmax 1/sum for both subs in one op (sum rows adjacent)
                    rs = attn.tile([1, 256], BF16, tag="rs", bufs=8)
                    with nc.allow_low_precision(reason="softmax 1/sum in bf16"):
                        nc.vector.reciprocal(rs[:], pavt[64:65, 0:256])
                    finalize_pair(p, qb, pavt, rs)

                # E/F one block behind D so its matmuls never stall the PE
                # queue on D(qb)'s normalization tail
                if qb >= 1:
                    emit_ef(qb - 1)
            emit_ef(3)

            # ---- Phase G: FFN (bf16) ----
            for m in range(16):
                pg = ps.tile([128, 512], F32, tag="pav", bufs=3, name="pg")
                for c in range(4):
                    nc.tensor.matmul(pg[:], w1s[c][:, :, m * 128:(m + 1) * 128],
                                     h2Tp[c][:, :, :],
                                     start=(c == 0), stop=(c == 3), perf_mode=DR)
                # gelu(pg / 64): undo the fp8 weight scale exactly
                nc.scalar.activation(gT[m][:], pg[:], AF.Gelu, scale=1.0 / WS)

            w2s = [wload(D, w2[k * 128:(k + 1) * 128, :], BF16) for k in range(16)]
            for t in range(4):
                ot = work.tile([128, D], F32, tag="ot", bufs=3)
                for nh in range(2):
                    po = ps.tile([128, 512], F32, tag="pa", bufs=2, name="po2")
                    for k in range(16):
                        nc.tensor.matmul(po[:], gT[k][:, t * 128:(t + 1) * 128],
                                         w2s[k][:, nh * 512:(nh + 1) * 512],
                                         start=(k == 0), stop=(k == 15))
                    nc.vector.tensor_tensor(out=ot[:, nh * 512:(nh + 1) * 512],
                                            in0=po[:], in1=x2[t][:, nh * 512:(nh + 1) * 512],
                                            op=ALU.add)
                    nc.sync.dma_start(out_d[t * 128:(t + 1) * 128, nh * 512:(nh + 1) * 512],
                                      ot[:, nh * 512:(nh + 1) * 512])

    _CACHED["nc"] = nc
    return nc


# ---------------------------------------------------------------------------
# host wrapper
# ---------------------------------------------------------------------------
def _pair8(w, scale):
    """[K, N] f32 -> [K//256, 128, 2*N] e4m3 DoubleRow pair layout."""
    f8 = ml_dtypes.float8_e4m3
    K, N = w.shape
    w8 = (np.asarray(w, np.float32) * scale).astype(f8)
    return np.ascontiguousarray(
        w8.reshape(K // 256, 2, 128, N).transpose(0, 2, 1, 3).reshape(K // 256, 128, 2 * N))


def _host_inputs(x, qkv_w, out_w, ffn_w1, ffn_w2):
    bf = ml_dtypes.bfloat16
    shared = {
        "wq8": _pair8(qkv_w, WS),
        "wo8": _pair8(out_w, WS),
        "w18": _pair8(ffn_w1, WS),
        "w2": np.ascontiguousarray(np.asarray(ffn_w2).astype(bf)),
        "ident": np.eye(128, dtype=bf),
    }
    r = np.arange(128)
    # transposed-score masks [key_local, query]: for query i, keys j in
    # [i, i+256] of the 384-band are valid
    t_lo = np.where(r[:, None] >= r[None, :], 0.0, NEG).astype(np.float32)
    t_hi = np.where(r[:, None] <= r[None, :], 0.0, NEG).astype(np.float32)
    zeros = np.zeros((128, 128), np.float32)
    full = np.full((128, 128), NEG, np.float32)

    def band(c0, c2):
        return np.concatenate([c0, zeros, c2], axis=1)

    in_maps = []
    for core in range(8):
        b, ck = core // 4, core % 4
        lo = ck * 512 - HALO
        xsl = np.zeros((R, D), np.float32)
        s, e = max(lo, 0), min(lo + R, L)
        xsl[s - lo:e - lo] = x[b, s:e]
        m_first = band(full if ck == 0 else t_lo, t_hi)
        m_mid = band(t_lo, t_hi)
        m_last = band(t_lo, full if ck == 3 else t_hi)
        in_maps.append({
            "xs": xsl,
            "maskd": np.stack([m_first, m_mid, m_last]).astype(bf),
            **shared,
        })
    return in_maps


def kernel(x, qkv_w, qkv_b, out_w, out_b, ln1_g, ln1_b, ln2_g, ln2_b,
           ffn_w1, ffn_b1, ffn_w2, ffn_b2, _return_results=False):
    x = np.asarray(x, np.float32)
    nc = _build_program()
    in_maps = _host_inputs(x, np.asarray(qkv_w), np.asarray(out_w),
                           np.asarray(ffn_w1), np.asarray(ffn_w2))
    res = run_bass_kernel_spmd(nc, in_maps, list(range(8)))
    out = np.empty((B, L, D), np.float32)
    for core in range(8):
        b, ck = core // 4, core % 4
        out[b, ck * 512:(ck + 1) * 512] = res.results[core]["out"]
    if _return_results:
        return out, res
    return out
